# revision 2
# baseline (speedup 1.0000x reference)
"""Involution-style aggregation (nn_AggregationNonCupy) on 8 Trainium2 NeuronCores.

out[n, g*32+cw, y, x] = sum_{i,j in 3x3} weight[n, cw, i*3+j, y*64+x]
                        * input[n, g*32+cw, y+i-1, x+j-1]        (zero padded)

Sharding: data-parallel over batch n (16 batches -> 2 per core).

Per-core design (v2, fp16 products + PE tap-accumulation):
  - partition dim packs (q, cw) = 4 spatial quarters x 32 weight channels;
    free dim packs (n_pair, 16 out rows x 64 cols).
  - input is DMA'd as fp32 halo tiles (18 rows/quarter), converted to TWO
    fp16 copies offset by one column so every tap's read is 4B-aligned
    (DVE tensor_tensor 2x_1P mode needs 16-bit dtype + aligned step-1 APs).
  - 9 DVE tensor_mul products per group (fp16, 2x mode), tap accumulation
    done on TensorE: identity matmul into PSUM fp32 (start/stop groups).
  - x-boundary taps are handled by zeroing the first/last column of the
    fp16 weight copies, so out-of-range tap reads multiply by zero.
  - ScalarE does fp32->fp16 weight/input conversion + PSUM evacuation;
    GPSIMD does the shifted input copy + memsets; DMA via HW DGE.
"""

import numpy as np

import concourse.bacc as bacc
import concourse.mybir as mybir
import concourse.tile as tile
from concourse.bass_utils import run_bass_kernel_spmd

# Problem constants (hardcoded per harness contract)
N_TOTAL, C_X, H, W = 16, 512, 64, 64
C_W = 32
N_CORES = 8
N_SH = N_TOTAL // N_CORES  # batches per core

TAPS = [(i, j) for i in range(3) for j in range(3)]
MM_N = 512  # max matmul free dim (one PSUM bank of fp32)


def emit_kernel(tc, x, wgt, o, *, n_sh, cx, h, w, reps=1):
    """Emit the tile program.

    x   : DRAM AP [n_sh, cx, h*w]    fp32 input
    wgt : DRAM AP [n_sh, 32, 9, h*w] fp32 weights
    o   : DRAM AP [n_sh, cx, h*w]    fp32 output
    reps: repeat the whole body in an on-device For_i (benchmarking only;
          the body is idempotent so the output is unchanged)
    """
    nc = tc.nc
    f32 = mybir.dt.float32
    f16 = mybir.dt.float16
    g_count = cx // C_W
    q = 4
    rq = h // q            # output rows per quarter
    lq = rq * w            # free-dim elements per quarter (one batch)
    blk = (rq + 2) * w + 2   # per-batch block: lead pad + (rq+2) rows + tail pad
    tcols = n_sh * blk       # fp32/fp16 input tile width (all batches)
    wblk = 9 * lq            # per-batch weight block

    # DRAM views
    wv = wgt.rearrange("n cw k (q c) -> n k q cw c", q=q)
    # output per (n, g): partition = (q, cw), free = c  (DMA APs max 3 dims)
    ov = o.rearrange("n (g cw) (q c) -> n g q cw c", cw=C_W, q=q)
    # input per (g, q): partition = cw, free = (n, cols)
    xg = x.rearrange("n (g cw) l -> g cw n l", cw=C_W)

    ident_dram = nc.inline_tensor(np.eye(128, dtype=np.float16), name="ident")

    with (
        tc.tile_pool(name="const", bufs=1) as const_pool,
        tc.tile_pool(name="wstage", bufs=3) as wstage_pool,
        tc.tile_pool(name="w16", bufs=1) as w16_pool,
        tc.tile_pool(name="inpool", bufs=3) as inpool,
        tc.tile_pool(name="in16", bufs=3) as in16_pool,
        tc.tile_pool(name="prodpool", bufs=9) as prodpool,
        tc.tile_pool(name="psumpool", bufs=2, space="PSUM") as psumpool,
        tc.tile_pool(name="outpool", bufs=4) as outpool,
    ):
        ident = const_pool.tile([128, 128], f16)
        nc.sync.dma_start(ident[:], ident_dram.ap())

        if reps == 1:
            _emit_body(tc, locals())
        else:
            with tc.For_i(0, reps, 1):
                _emit_body(tc, locals())


def _emit_body(tc, env):
    nc = env["nc"]
    f32, f16 = env["f32"], env["f16"]
    n_sh, g_count, q, rq, lq = (env["n_sh"], env["g_count"], env["q"],
                                env["rq"], env["lq"])
    blk, tcols, wblk, w, h = (env["blk"], env["tcols"], env["wblk"],
                              env["w"], env["h"])
    wv, ov, xg = env["wv"], env["ov"], env["xg"]
    ident = env["ident"]
    wstage_pool, w16_pool, inpool = (env["wstage_pool"], env["w16_pool"],
                                     env["inpool"])
    in16_pool, prodpool, psumpool, outpool = (env["in16_pool"], env["prodpool"],
                                              env["psumpool"], env["outpool"])
    if True:
        weight_phase = []

        # ---- weights: stage fp32 in 3-tap batches, convert to one resident
        # fp16 tile (batching shrinks the startup serial chain)
        wt16 = w16_pool.tile([128, n_sh * wblk], f16)

        def load_weights():
            # k-batch outer, n inner, boundary memsets right after each
            # conversion: the first products read BOTH n-blocks of wt16, so
            # this ordering makes tap k usable after ~2 conversions instead
            # of after the whole weight phase
            for kb in range(0, 9, 3):
                for n in range(n_sh):
                    ws = wstage_pool.tile([128, 3 * lq], f32, tag="ws")
                    for dk in range(3):
                        nc.sync.dma_start(ws[:, dk * lq:(dk + 1) * lq],
                                          wv[n, kb + dk])
                    nc.scalar.copy(
                        wt16[:, n * wblk + kb * lq:n * wblk + (kb + 3) * lq],
                        ws[:])
                    for k in range(kb, kb + 3):
                        i, j = TAPS[k]
                        if j == 1:
                            continue
                        wk = wt16[:, n * wblk + k * lq:n * wblk + (k + 1) * lq]
                        wk = wk.rearrange("p (y xx) -> p y xx", xx=w)
                        col = 0 if j == 0 else w - 1
                        nc.gpsimd.memset(wk[:, :, col:col + 1], 0.0)

        PIPE = 2
        stage_tiles = {}

        def input_stage(g):
            # ---- fp32 input tile: per batch block [pad, 18 rows, pad]
            it = inpool.tile([128, tcols], f32, tag="it")
            # pads: col 0 of each block and tail col of each block are adjacent
            # (blk-1, blk) except at the ends
            nc.gpsimd.memset(it[:, 0:1], 0.0)
            for n in range(1, n_sh):
                nc.gpsimd.memset(it[:, n * blk - 1:n * blk + 1], 0.0)
            nc.gpsimd.memset(it[:, tcols - 1:tcols], 0.0)
            itv = it.rearrange("p (n c) -> p n c", n=n_sh)
            # top halo row (q=0), bottom halo row (q=3)
            nc.gpsimd.memset(itv[0:C_W, :, 1:1 + w], 0.0)
            nc.gpsimd.memset(itv[3 * C_W:128, :, 1 + (rq + 1) * w:1 + (rq + 2) * w], 0.0)
            # interior loads: rows q*rq-1 .. q*rq+rq (clipped)
            nc.sync.dma_start(itv[0:C_W, :, 1 + w:blk - 1],
                              xg[g, :, :, 0:(rq + 1) * w])
            for qq in range(1, q - 1):
                nc.sync.dma_start(
                    itv[qq * C_W:(qq + 1) * C_W, :, 1:blk - 1],
                    xg[g, :, :, (qq * rq - 1) * w:(qq * rq + rq + 1) * w])
            nc.sync.dma_start(itv[(q - 1) * C_W:128, :, 1:1 + (rq + 1) * w],
                              xg[g, :, :, ((q - 1) * rq - 1) * w:h * w])

            # ---- fp16 copies: A (same columns), B (shifted +1 column so the
            # j=1 taps read 4B-aligned). ita conversion runs on ACT, which is
            # safe only because input stages are EMITTED ahead of compute
            # stages, so conv(g) sits ahead of evac(g-PIPE) in ACT's in-order
            # queue rather than behind evac(g-1).
            ita = in16_pool.tile([128, tcols], f16, tag="ita")
            nc.scalar.copy(ita[:], it[:])
            itb = in16_pool.tile([128, tcols], f16, tag="itb")
            nc.gpsimd.tensor_copy(itb[:, 1:tcols], it[:, 0:tcols - 1])
            stage_tiles[g] = (ita, itb)

        def compute_stage(g):
            ita, itb = stage_tiles.pop(g)
            # ---- products (fp16 2x) + PE tap accumulation into PSUM
            ps = psumpool.tile([128, n_sh * lq], f32, tag="ps")
            n_ch = (n_sh * lq + MM_N - 1) // MM_N
            for k, (i, j) in enumerate(TAPS):
                pk = prodpool.tile([128, n_sh * lq], f16, tag="prod")
                pkv = pk.rearrange("p (n c) -> p n c", n=n_sh)
                wk = wt16.rearrange("p (n c) -> p n c", n=n_sh)[:, :,
                                                              k * lq:(k + 1) * lq]
                off = i * w + j
                if j == 1:
                    src = itb.rearrange("p (n c) -> p n c", n=n_sh)[
                        :, :, off + 1:off + 1 + lq]
                else:
                    src = ita.rearrange("p (n c) -> p n c", n=n_sh)[
                        :, :, off:off + lq]
                if k == 8:
                    # offload the last tap's product to GPSIMD: it's ready
                    # long before its (stop) matmul is reachable, and it
                    # takes ~1 tap of work off the DVE critical path
                    nc.gpsimd.tensor_mul(pkv, wk, src)
                else:
                    nc.vector.tensor_mul(pkv, wk, src)
                for hh in range(n_ch):
                    sl = slice(hh * MM_N, min((hh + 1) * MM_N, n_sh * lq))
                    nc.tensor.matmul(ps[:, sl], ident[:], pk[:, sl],
                                     start=(k == 0), stop=(k == 8))

            ot = outpool.tile([128, n_sh * lq], f32, tag="ot")
            nc.scalar.copy(ot[:], ps[:])
            # issue output DMAs from the ACT HWDGE queue: they depend on the
            # evac that just ran on ACT, so they never head-of-line-block the
            # input DMAs on the SP queue
            for n in range(n_sh):
                nc.scalar.dma_start(ov[n, g], ot[:, n * lq:(n + 1) * lq])

        # emit the first input stages BEFORE the weight phase: the weight
        # DMAs (9.4 MB) otherwise sit ahead of the first input tiles in the
        # in-order SP queue and stall the compute pipeline ~30us at startup
        for g in range(min(PIPE, g_count)):
            input_stage(g)
        load_weights()
        for g in range(g_count):
            if g + PIPE < g_count:
                input_stage(g + PIPE)
            compute_stage(g)


def build_program(n_sh=N_SH, cx=C_X, h=H, w=W, reps=1):
    nc = bacc.Bacc("TRN2", target_bir_lowering=False, debug=False,
                   enable_asserts=True, num_devices=N_CORES)
    f32 = mybir.dt.float32
    x = nc.dram_tensor("x", [n_sh, cx, h * w], f32, kind="ExternalInput").ap()
    wgt = nc.dram_tensor("w", [n_sh, C_W, 9, h * w], f32, kind="ExternalInput").ap()
    o = nc.dram_tensor("o", [n_sh, cx, h * w], f32, kind="ExternalOutput").ap()
    with tile.TileContext(nc) as tc:
        emit_kernel(tc, x, wgt, o, n_sh=n_sh, cx=cx, h=h, w=w, reps=reps)
    nc.compile()
    return nc


_CACHED_NC = None


def _get_nc():
    global _CACHED_NC
    if _CACHED_NC is None:
        _CACHED_NC = build_program()
    return _CACHED_NC


def make_in_maps(inputs):
    inp = np.ascontiguousarray(np.asarray(inputs["input"], dtype=np.float32))
    wgt = np.ascontiguousarray(np.asarray(inputs["weight"], dtype=np.float32))
    assert inp.shape == (N_TOTAL, C_X, H, W)
    assert wgt.shape == (N_TOTAL, C_W, 9, H * W)
    in_maps = []
    for c in range(N_CORES):
        sl = slice(c * N_SH, (c + 1) * N_SH)
        in_maps.append({
            "x": np.ascontiguousarray(inp[sl].reshape(N_SH, C_X, H * W)),
            "w": np.ascontiguousarray(wgt[sl]),
        })
    return in_maps


def assemble_output(res):
    return np.concatenate(
        [res.results[c]["o"].reshape(N_SH, C_X, H, W) for c in range(N_CORES)],
        axis=0)


def run(inputs, trace=False):
    """Run on 8 cores; returns (output [16,512,64,64] fp32, BassKernelResults)."""
    nc = _get_nc()
    in_maps = make_in_maps(inputs)
    res = run_bass_kernel_spmd(nc, in_maps, core_ids=list(range(N_CORES)),
                               trace=trace)
    return assemble_output(res), res


def kernel(**inputs):
    out, _ = run(inputs)
    return out

